# revision 16
# baseline (speedup 1.0000x reference)
"""HGNN conv kernel for Trainium2, data-parallel over time across 8 cores.

Per core (t = core index): out_b = Dv^-1/2 Gc De^-1 Gc^T Dv^-1/2 (x_b W + 1 b^T)
computed in factored form (L never materialized):
  Gs  = Dv^-1/2 Gc                      [N, E]
  z   = x_t^T Gs  per 128-row bf block  [BF, E]   (MM1)
  v   = zT-blocks @ blockdiag(W,W) + u0 bias^T  [E, BF]  (W-MM, fused bias)
  out = Gsd^T v with Gsd = de * Gs^T    [N, BF]   (MM2)
All big matmuls run in bf16 (rel err ~5e-3, PE at 1 cyc/row + fast
weight loads); stats (degree vectors) stay f32/f32r.

DMA layout: the on-chip n axis is stored as n = 8*p + k (p = partition,
k = 0..7 inner) so every HBM x/out descriptor moves a contiguous
(8 n x 64 f) = 2 KB run (vs 256 B in the naive n-major layout).
Matmul contraction over n is permutation-invariant, so only G/Gs/Gsd
need the matching row order. x is staged [p, b, k, f] (HWDGE, big
descriptors) and repacked on-chip to k-major bf16 for single-free-dim
lhsT APs. The emission order software-pipelines: stats -> MM1 m-tiles
(consuming x chunks as they land) with gsd transposes interleaved ->
MM2 blocks as their v columns complete -> pipelined stores.
"""

import sys

import numpy as np

sys.path.insert(0, "/opt/trn_rl_repo")

from contextlib import ExitStack

import concourse.bass as bass
import concourse.mybir as mybir
import concourse.tile as tile
from concourse import bacc, bass_utils
from concourse.masks import make_identity

P = 128
T = 8
B = 28          # batch entries per core
N = 1024        # nodes
E = 512         # hyperedges (256 static + 256 dynamic)
F = 64          # features
BF = B * F      # 1792
EPS = 1e-6
KN = 8          # inner n factor: n = 8*p + k
MT = BF // P    # 14 bf-tiles (2 batch entries each)
ET = E // P     # 4 e-tiles
NB = 4          # output free-dim chunks
NBW = BF // NB  # 448 = 7 batch entries * 64
XC = 14         # x load chunks (2 b-entries each, = 1 m-tile)

f32 = mybir.dt.float32
f32r = mybir.dt.float32r
bf16 = mybir.dt.bfloat16


def _build_nc():
    nc = bacc.Bacc("TRN2", target_bir_lowering=False, debug=False)

    xs = nc.dram_tensor("xs", [B, N, F], f32, kind="ExternalInput").ap()
    g = nc.dram_tensor("g", [N, 256], f32, kind="ExternalInput").ap()
    g1 = nc.dram_tensor("g1", [N, 256], f32, kind="ExternalInput").ap()
    w = nc.dram_tensor("w", [F, F], f32, kind="ExternalInput").ap()
    bvec = nc.dram_tensor("b", [F], f32, kind="ExternalInput").ap()
    os_ = nc.dram_tensor("os", [B, N, F], f32, kind="ExternalOutput").ap()

    with tile.TileContext(nc) as tc, ExitStack() as ctx:
        const = ctx.enter_context(tc.tile_pool(name="const", bufs=1))
        big = ctx.enter_context(tc.tile_pool(name="big", bufs=1))
        ztp = ctx.enter_context(tc.tile_pool(name="ztp", bufs=3))
        xstage = ctx.enter_context(tc.tile_pool(name="xstage", bufs=4))
        ps_warm = ctx.enter_context(tc.tile_pool(name="ps_warm", bufs=1, space="PSUM"))
        ps_small = ctx.enter_context(tc.tile_pool(name="ps_small", bufs=2, space="PSUM"))
        ps_z = ctx.enter_context(tc.tile_pool(name="ps_z", bufs=2, space="PSUM"))
        ps_o = ctx.enter_context(tc.tile_pool(name="ps_o", bufs=3, space="PSUM"))

        # ---- input loads -------------------------------------------------
        # G rows n = 8p+k: per partition one 8 KB contiguous DRAM run.
        # HWDGE f32, then DVE casts to bf16 for the matmul paths.
        H = KN // 2
        gcs32 = big.tile([P, KN, 256], f32, name="gcs32")
        gcd32 = big.tile([P, KN, 256], f32, name="gcd32")
        gr = g.rearrange("(p k) e -> p k e", k=KN)
        g1r = g1.rearrange("(p k) e -> p k e", k=KN)
        xs_r = xs.rearrange("b (p k) f -> p b k f", k=KN)
        stage_bufs = [
            xstage.tile([P, 2, KN, F], f32, name="xst") for _ in range(XC)
        ]
        # Ring split: gcs + all x chunks on the sync ring (sync engine is
        # otherwise idle, so its serial ~0.7us/dispatch cost is free);
        # gcd + small consts on the scalar ring so both G tiles stream
        # concurrently and the ACT engine stays free for early casts.
        nc.sync.dma_start(gcs32[:], gr[:])
        nc.scalar.dma_start(gcd32[:], g1r[:])
        for c in range(XC):
            nc.sync.dma_start(stage_bufs[c][:], xs_r[:, 2 * c : 2 * c + 2])
        xp = big.tile([P, KN, B, F], bf16, name="xp")

        def repack(c, eng):
            # k-major bf16 repack; the f32->bf16 cast rides the copy
            dst = xp[:, :, 2 * c : 2 * c + 2, :]
            srcv = stage_bufs[c][:].rearrange("p b k f -> p k b f")
            if eng == "v":
                nc.vector.tensor_copy(dst, srcv)
            else:
                nc.scalar.copy(dst, srcv)

        # ---- constants ---------------------------------------------------
        bdw_f = const.tile([P, P], f32, name="bdw_f")
        nc.vector.memset(bdw_f[:], 0.0)
        nc.scalar.dma_start(bdw_f[0:64, 0:64], w)
        nc.scalar.dma_start(bdw_f[64:128, 64:128], w)
        bdw = const.tile([P, P], bf16, name="bdw")
        nc.vector.tensor_copy(bdw[:], bdw_f[:])

        btmp = const.tile([1, F], f32, name="btmp")
        nc.scalar.dma_start(btmp[:], bvec[None, :])
        bias2 = const.tile([1, 2, F], f32r, name="bias2")
        nc.vector.tensor_copy(bias2[:], btmp[0:1, None, :].to_broadcast([1, 2, F]))
        bias_bc = const.tile([P, P], f32r, name="bias_bc")
        nc.gpsimd.partition_broadcast(
            bias_bc[:], bias2[:].rearrange("o t f -> o (t f)")
        )

        ident_f = const.tile([P, P], f32, name="ident_f")
        make_identity(nc, ident_f[:])
        ident = const.tile([P, P], f32r, name="ident")
        nc.vector.tensor_copy(ident[:], ident_f[:])
        ident16 = const.tile([P, P], bf16, name="ident16")
        nc.vector.tensor_copy(ident16[:], ident_f[:])

        warm_f = const.tile([P, E], f32, name="warm_f")
        nc.vector.memset(warm_f[:], 0.0)

        # PE warmup: gated on the first G half arriving, so the HAM
        # un-throttle overlaps the load head and ends right as the real
        # matmul stream begins. Values are junk (never read).
        warm_r = const.tile([P, E], f32r, name="warm_r")
        nc.vector.scalar_tensor_tensor(
            out=warm_r[:], in0=gcs32[:, 0:H].rearrange("p k e -> p (k e)")[:, 0:E],
            scalar=0.0, in1=warm_f[:],
            op0=mybir.AluOpType.mult, op1=mybir.AluOpType.add,
        )
        warm_ps = ps_warm.tile([P, E], f32, name="warm_ps")
        for _ in range(8):
            nc.tensor.matmul(warm_ps[:], warm_r[:, 0:128], warm_r[:], start=True, stop=True)

        # ---- degree stats, per G half ------------------------------------
        gcs = big.tile([P, KN, 256], bf16, name="gcs")
        gcd = big.tile([P, KN, 256], bf16, name="gcd")
        rs = const.tile([P, KN], f32, name="rs")
        rs2 = const.tile([P, KN], f32, name="rs2")
        eps_col = const.tile([P, 1], f32, name="eps_col")
        nc.vector.memset(eps_col[:], EPS)
        sq = const.tile([P, KN], f32, name="sq")
        dv = const.tile([P, KN], f32, name="dv")
        onesdv_f = const.tile([P, KN, 2], f32, name="onesdv_f")
        nc.vector.memset(onesdv_f[:, :, 0:1], 1.0)
        onesdv = const.tile([P, KN, 2], bf16, name="onesdv")
        gs_all = big.tile([P, KN, E], bf16, name="gs_all")

        nc.scalar.copy(gcs[:], gcs32[:])
        nc.scalar.copy(gcd[:], gcd32[:])
        # dv = 1/sqrt(rowsum(Gc) + eps)
        nc.vector.reduce_sum(rs[:, :, None], gcs32[:], axis=mybir.AxisListType.X)
        nc.vector.reduce_sum(rs2[:, :, None], gcd32[:], axis=mybir.AxisListType.X)
        nc.vector.scalar_tensor_tensor(
            out=rs[:], in0=rs[:], scalar=1.0, in1=rs2[:],
            op0=mybir.AluOpType.mult, op1=mybir.AluOpType.add,
        )
        nc.scalar.activation(
            sq[:], rs[:], mybir.ActivationFunctionType.Sqrt, bias=eps_col[:]
        )
        nc.vector.reciprocal(dv[:], sq[:])
        nc.vector.tensor_copy(onesdv_f[:, :, 1:2], dv[:, :, None])
        nc.vector.tensor_copy(onesdv[:], onesdv_f[:])
        # first x repacks on DVE right after the dv chain
        repack(0, "v")
        repack(1, "v")
        # Gs = dv * Gc (bf16, straight from the f32 tiles)
        for k in range(KN):
            nc.vector.tensor_scalar(
                out=gs_all[:, k, 0:256], in0=gcs32[:, k, :], scalar1=dv[:, k : k + 1],
                scalar2=None, op0=mybir.AluOpType.mult,
            )
            nc.vector.tensor_scalar(
                out=gs_all[:, k, 256:512], in0=gcd32[:, k, :], scalar1=dv[:, k : k + 1],
                scalar2=None, op0=mybir.AluOpType.mult,
            )

        # colsums of Gc (row 0) and Gs (row 1) -> [2, E], emitted per half.
        # The two accumulation groups live in SEPARATE PSUM banks: a
        # start=True on one group clears has_written for its whole bank,
        # so interleaved groups must not share one.
        stats_s = ps_small.tile([2, 256], f32, name="sp")
        stats_d = ps_small.tile([2, 256], f32, name="sp")

        def colsums(h):
            for k in range(h * H, h * H + H):
                nc.tensor.matmul(
                    stats_s[:], onesdv[:, k, :], gcs[:, k, :],
                    start=(k == 0), stop=(k == KN - 1),
                )
                nc.tensor.matmul(
                    stats_d[:], onesdv[:, k, :], gcd[:, k, :],
                    start=(k == 0), stop=(k == KN - 1),
                )

        stats_sb = const.tile([2, E], f32r, name="stats_sb")
        statsT = const.tile([P, ET, 2], f32, name="statsT")
        de_col = const.tile([P, ET], f32, name="de_col")

        def stats_finish():
            nc.vector.tensor_copy(stats_sb[:, 0:256], stats_s[:])
            nc.vector.tensor_copy(stats_sb[:, 256:512], stats_d[:])
            for j in range(ET):
                tp = ps_small.tile([P, P], f32r, name="sp")[:, 0:2]
                nc.tensor.matmul(
                    tp[:], stats_sb[:, j * P : (j + 1) * P], ident[0:2, 0:2],
                    is_transpose=True,
                )
                nc.vector.tensor_copy(statsT[:, j, :], tp[:])
            nc.vector.tensor_scalar(
                out=de_col[:], in0=statsT[:, :, 0], scalar1=EPS, scalar2=None,
                op0=mybir.AluOpType.add,
            )
            nc.vector.reciprocal(de_col[:], de_col[:])

        # ---- main pipeline ----------------------------------------------
        gsd_all = big.tile([P, ET, KN, P], bf16, name="gsd_all")
        v_all = big.tile([P, ET, BF], bf16, name="v_all")
        os_all = big.tile([P, B, KN, F], f32, name="os_all")
        os_r = os_.rearrange("b (p k) f -> p b k f", k=KN)

        def gsd_block(j):
            # Gsd[e, (j, k, p)] = de[e] * Gs[(p, k), e] via PE transpose
            for k in range(KN):
                tp = ps_small.tile([P, P], bf16, name="sp")
                nc.tensor.matmul(
                    tp[:], gs_all[:, k, j * P : (j + 1) * P], ident16[:],
                    is_transpose=True,
                )
                if k % 2 == 0:
                    nc.vector.tensor_scalar(
                        out=gsd_all[:, j, k, :], in0=tp[:],
                        scalar1=de_col[:, j : j + 1], scalar2=None,
                        op0=mybir.AluOpType.mult,
                    )
                else:
                    nc.scalar.activation(
                        gsd_all[:, j, k, :], tp[:],
                        mybir.ActivationFunctionType.Copy,
                        scale=de_col[:, j : j + 1],
                    )

        def mm1_mms(m, zps, k0, k1):
            for k in range(k0, k1):
                xm = xp[:, k, 2 * m : 2 * m + 2, :].rearrange("p b f -> p (b f)")
                nc.tensor.matmul(
                    zps[:], xm, gs_all[:, k, :],
                    start=(k == 0), stop=(k == KN - 1),
                )

        def mm1_tile(m, zps=None):
            if zps is None:
                zps = ps_z.tile([P, E], f32, name="zps")
                mm1_mms(m, zps, 0, KN)
            zt = ztp.tile([P, E], bf16, name="zt")
            nc.scalar.copy(zt[:], zps[:])
            for j in range(ET):
                wps = ps_small.tile([P, P], f32, name="sp")
                nc.tensor.matmul(
                    wps[:], zt[:, j * P : (j + 1) * P], bdw[:],
                    start=True, stop=True,
                )
                # v = (bias_bcast * u0_col) + zw_psum, rounded to bf16
                nc.vector.scalar_tensor_tensor(
                    out=v_all[:, j, m * P : (m + 1) * P],
                    in0=bias_bc[:],
                    scalar=statsT[:, j, 1:2],
                    in1=wps[:],
                    op0=mybir.AluOpType.mult,
                    op1=mybir.AluOpType.add,
                )

        def mm2_block(nb, split_store):
            # out chunk [128 p, 7 b, 64 f] per k; store via 2-KB runs
            for k in range(KN):
                ops = ps_o.tile([P, NBW], f32, name="ops")
                for j in range(ET):
                    nc.tensor.matmul(
                        ops[:], gsd_all[:, j, k, :],
                        v_all[:, j, nb * NBW : (nb + 1) * NBW],
                        start=(j == 0), stop=(j == ET - 1),
                    )
                dst = os_all[:, nb * 7 : (nb + 1) * 7, k, :]
                src = ops[:].rearrange("p (c f) -> p c f", f=F)
                if k % 2 == 0:
                    nc.vector.tensor_copy(dst, src)
                else:
                    nc.scalar.copy(dst, src)
                if split_store and k % 2 == 1 and k < KN - 1:
                    nc.scalar.dma_start(
                        os_r[:, nb * 7 : (nb + 1) * 7, k - 1 : k + 1],
                        os_all[:, nb * 7 : (nb + 1) * 7, k - 1 : k + 1],
                    )
            if split_store:
                nc.scalar.dma_start(
                    os_r[:, nb * 7 : (nb + 1) * 7, KN - 2 :],
                    os_all[:, nb * 7 : (nb + 1) * 7, KN - 2 :],
                )
            else:
                nc.scalar.dma_start(
                    os_r[:, nb * 7 : (nb + 1) * 7], os_all[:, nb * 7 : (nb + 1) * 7]
                )

        mm2_after = {3: 0, 7: 1, 10: 2, 13: 3}  # m-tile -> ready nb block

        colsums(0)
        colsums(1)
        stats_finish()
        mm1_tile(0)
        gsd_block(0)
        for m in range(1, MT):
            if m >= 2:
                repack(m, "v" if m % 2 else "s")
            mm1_tile(m)
            if m < ET:
                gsd_block(m)
            if m in mm2_after:
                nb = mm2_after[m]
                mm2_block(nb, split_store=(nb == NB - 1))

    nc.finalize()
    return nc


_NC = None


def _get_nc():
    global _NC
    if _NC is None:
        _NC = _build_nc()
    return _NC


def kernel(x, G, G1, weight, bias):
    nc = _get_nc()
    x = np.ascontiguousarray(x, dtype=np.float32)
    G = np.ascontiguousarray(G, dtype=np.float32)
    G1 = np.ascontiguousarray(G1, dtype=np.float32)
    weight = np.ascontiguousarray(weight, dtype=np.float32)
    bias = np.ascontiguousarray(bias, dtype=np.float32)

    in_maps = []
    for c in range(T):
        in_maps.append(
            {
                "xs": x[c * B : (c + 1) * B],
                "g": G,
                "g1": np.ascontiguousarray(G1[c]),
                "w": weight,
                "b": bias,
            }
        )
    res = bass_utils.run_bass_kernel_spmd(nc, in_maps, core_ids=list(range(T)))
    return np.concatenate([r["os"] for r in res.results], axis=0)
